# revision 6
# baseline (speedup 1.0000x reference)
"""Depth-upsample module kernel for 8 TRN2 NeuronCores.

Pipeline per core (1/8 of batch*height):
  conv1 3x3 8->8 + bias + relu   (PE banded-dy bf16 matmuls, 3 per block)
  conv2 1x1 8->36 (raw)          (PE bf16, 1 matmul per subpixel ab)
  EP = [exp(0.25*conv2+0.25*b2) | E*unfold(depth)]   (ACT exp | DVE mul, bf16)
  [Den|Num] = band @ EP          (PE bf16, 3 chunk matmuls per ab -> psum quad)
  out = Num * recip_approx(Den)  (DVE), b-interleaved in SBUF, contiguous DMA

Layout: row-blocks of R=14 output rows; SBUF partitions pack (row, channel):
  conv input  xb   [(r16,i8)=128, 642]   bf16
  conv1 out   Y    [(r14,o8)=112, 640]   bf16
  EP               [(r14,k9)=126, 1280]  bf16: E cols 0:640, P cols 640:1280
  psumND           [128, 1280] f32: quadrant ab rows 32ab..+14, Den|Num cols
  out interleave   OI[64a+r, 2x+b] f32, rows of 2W -> contiguous DMA
PSUM budget: ps12 pool 2x2 banks (conv1/conv2 rotate) + nd 3 banks = 7 of 8.
"""

import numpy as np
import ml_dtypes

H, W = 512, 640
N_IMG, C_IN = 4, 8
HALF = H // 2           # rows per core (shard = image x half)
RB = 14                 # output rows per block
WP = W + 2              # padded width
CWB = 854               # bf16 const cols: w1(336) w2(504) band(14)
CWF = 5                 # f32 const cols: b1(1) b2(4)


def _build_consts(conv1_w, conv1_b, conv2_w, conv2_b):
    f32 = np.float32
    bf16 = ml_dtypes.bfloat16
    # lhsT1[dx, (r,i), (r',o)] = W1[o,i,r-r',dx] for r-r' in {0,1,2}
    lhsT1 = np.zeros((3, 128, 112), f32)
    for dx in range(3):
        for rp in range(14):
            for dy in range(3):
                r = rp + dy
                lhsT1[dx, r * 8:(r + 1) * 8, rp * 8:(rp + 1) * 8] = \
                    conv1_w[:, :, dy, dx].T  # [i, o]
    # lhsT2[ab, (r,i), (r,k)] = W2[4k+ab, i]
    lhsT2 = np.zeros((4, 112, 126), f32)
    w2 = conv2_w[:, :, 0, 0]  # [36, 8]
    for ab in range(4):
        for r in range(14):
            for k in range(9):
                lhsT2[ab, r * 8:(r + 1) * 8, r * 9 + k] = w2[k * 4 + ab, :]
    # band[(r,k), r'] = 1 iff r == r'
    band = np.zeros((126, 14), f32)
    for r in range(14):
        band[r * 9:(r + 1) * 9, r] = 1
    cstb = np.zeros((128, CWB), bf16)
    for dx in range(3):
        cstb[:, 112 * dx: 112 * (dx + 1)] = lhsT1[dx].astype(bf16)
    for ab in range(4):
        cstb[:112, 336 + 126 * ab: 336 + 126 * (ab + 1)] = lhsT2[ab].astype(bf16)
    cstb[:126, 840:854] = band.astype(bf16)
    cstf = np.zeros((128, CWF), f32)
    cstf[:112, 0] = np.tile(conv1_b.astype(f32), 14)
    for ab in range(4):
        for r in range(14):
            for k in range(9):
                cstf[r * 9 + k, 1 + ab] = 0.25 * float(conv2_b[k * 4 + ab])
    return cstb, cstf


def _build_bass():
    import concourse.bass as bass
    import concourse.bacc as bacc
    import concourse.tile as tile
    from concourse import mybir

    f32 = mybir.dt.float32
    bf16 = mybir.dt.bfloat16
    nc = bacc.Bacc(None, target_bir_lowering=False)

    X = nc.dram_tensor("xh", [C_IN, HALF + 2, WP], bf16, kind="ExternalInput")
    DUNF = nc.dram_tensor("dunf", [HALF * 9, W], bf16, kind="ExternalInput")
    CONSTB = nc.dram_tensor("cstb", [128, CWB], bf16, kind="ExternalInput")
    CONSTF = nc.dram_tensor("cstf", [128, CWF], f32, kind="ExternalInput")
    OUT = nc.dram_tensor("out", [2 * HALF, 2 * W], f32, kind="ExternalOutput")

    nblocks = (HALF + RB - 1) // RB  # 19 (last block R=4)

    with tile.TileContext(nc) as tc:
        with (
            tc.tile_pool(name="consts", bufs=1) as consts,
            tc.tile_pool(name="xp", bufs=3) as xp,
            tc.tile_pool(name="dp", bufs=3) as dp,
            tc.tile_pool(name="yp", bufs=2) as yp,
            tc.tile_pool(name="ep", bufs=6) as ep,
            tc.tile_pool(name="op", bufs=3) as op,
            tc.tile_pool(name="scr", bufs=2) as scr,
            tc.tile_pool(name="ps12", bufs=2, space="PSUM") as ps12,
            tc.tile_pool(name="nd", bufs=1, space="PSUM") as nd,
        ):
            cstb = consts.tile([128, CWB], bf16, tag="cstb")
            nc.sync.dma_start(out=cstb, in_=CONSTB[:])
            cstf = consts.tile([128, CWF], f32, tag="cstf")
            nc.sync.dma_start(out=cstf, in_=CONSTF[:])
            w1t = [cstb[:, 112 * dx: 112 * (dx + 1)] for dx in range(3)]
            w2t = [cstb[:112, 336 + 126 * ab: 336 + 126 * (ab + 1)]
                   for ab in range(4)]
            bandt = cstb[:126, 840:854]
            b1t = cstf[:112, 0:1]
            b2t = [cstf[:126, 1 + ab: 2 + ab] for ab in range(4)]
            # consume the const-DMA ticks (keeps real ops at <=1 wait)
            nc.tensor.ldweights(cstb[:1, :2])
            scrf = scr.tile([1, 1], f32, tag="scrf")
            nc.vector.tensor_copy(scrf, cstf[:1, :1])

            for b in range(nblocks):
                R = min(RB, HALF - RB * b)
                Rin = R + 2
                s = RB * b
                kp = R * 9   # partitions in (r,k) tiles
                yq = R * 8   # partitions in (r,o) tiles

                # --- load conv input block [(r,i), w] bf16 ---
                xb = xp.tile([128, WP], bf16, tag="xb")
                x_in = bass.AP(
                    tensor=X[:].tensor, offset=s * WP,
                    ap=[[WP, Rin], [(HALF + 2) * WP, C_IN], [1, WP]],
                )
                nc.sync.dma_start(out=xb[: Rin * 8], in_=x_in)

                # --- load unfolded depth [(r,k), x] bf16 (host-prepared) ---
                dunf = dp.tile([126, W], bf16, tag="dunf")
                nc.sync.dma_start(out=dunf[:kp], in_=DUNF[9 * s: 9 * s + kp])
                scrap = scr.tile([1, 1], bf16, tag="scrap")
                nc.vector.tensor_copy(scrap, dunf[:1, :1])  # eat DMA tick

                # --- conv1: 3 dx matmuls x 2 col chunks -> psum1 ---
                nc.tensor.ldweights(xb[:1, :2])  # eat DMA tick
                psum1 = ps12.tile([128, W], f32, tag="ps")
                for c0, cn in ((0, 512), (512, 128)):
                    for dx in range(3):
                        nc.tensor.matmul(
                            psum1[:yq, c0:c0 + cn],
                            w1t[dx][: Rin * 8, :yq],
                            xb[: Rin * 8, dx + c0: dx + c0 + cn],
                            start=(dx == 0), stop=(dx == 2),
                        )

                # --- bias+relu -> Y (SBUF bf16) ---
                Y = yp.tile([112, W], bf16, tag="y")
                nc.scalar.activation(
                    out=Y[:yq], in_=psum1[:yq],
                    func=mybir.ActivationFunctionType.Relu,
                    bias=b1t[:yq], scale=1.0,
                )
                nc.tensor.ldweights(Y[:1, :2])  # eat ACT tick

                # --- per ab: conv2 -> exp -> x depth -> band-reduce [D|N] ---
                psumND = nd.tile([128, 2 * W], f32, tag="psumnd")
                for ab in range(4):
                    psum2 = ps12.tile([128, W], f32, tag="ps")
                    for c0, cn in ((0, 512), (512, 128)):
                        nc.tensor.matmul(
                            psum2[:kp, c0:c0 + cn],
                            w2t[ab][:yq, :kp],
                            Y[:yq, c0:c0 + cn],
                            start=True, stop=True,
                        )
                    EP = ep.tile([126, 2 * W], bf16, tag="ep")
                    nc.scalar.activation(
                        out=EP[:kp, 0:W], in_=psum2[:kp],
                        func=mybir.ActivationFunctionType.Exp,
                        bias=b2t[ab][:kp], scale=0.25,
                    )
                    nc.vector.tensor_mul(
                        EP[:kp, W:2 * W], EP[:kp, 0:W], dunf[:kp])
                    # band-reduce 9 taps: [Den|Num] -> psum quadrant ab
                    for c0, cn in ((0, 512), (512, 512), (1024, 256)):
                        nc.tensor.matmul(
                            psumND[32 * ab: 32 * ab + R, c0:c0 + cn],
                            bandt[:kp, :R], EP[:kp, c0:c0 + cn],
                            start=True, stop=True,
                            tile_position=(0, 32 * ab),
                        )

                # --- drain psumND fast: Num->SBUF on ACT, recip(Den) on DVE
                NI = op.tile([128, W], f32, tag="ni")
                nc.scalar.activation(
                    out=NI, in_=psumND[:, W:2 * W],
                    func=mybir.ActivationFunctionType.Copy, scale=1.0)
                RD = op.tile([128, W], f32, tag="rd")
                nc.vector.reciprocal_approx_fast(out=RD, in_=psumND[:, 0:W])
                # divide; interleave b into contiguous 2W rows (GPSIMD, idle)
                OI = op.tile([128, 2 * W], f32, tag="oi")
                for ab in range(4):
                    a, bb = ab >> 1, ab & 1
                    nc.gpsimd.tensor_mul(
                        OI[64 * a: 64 * a + R, bb::2],
                        NI[32 * ab: 32 * ab + R],
                        RD[32 * ab: 32 * ab + R])

                # --- store: out[2(s+r)+a, :] = OI[64a+r, :]  (contiguous rows)
                for a in range(2):
                    o_out = bass.AP(
                        tensor=OUT[:].tensor,
                        offset=(2 * s + a) * (2 * W),
                        ap=[[4 * W, R], [1, 2 * W]],
                    )
                    nc.sync.dma_start(out=o_out, in_=OI[64 * a: 64 * a + R])

    nc.compile()
    return nc


_NC_CACHE = None


def prep_inputs(depth, cost_volume, conv1_w, conv1_b, conv2_w, conv2_b):
    bf16 = ml_dtypes.bfloat16
    depth = np.asarray(depth, np.float32)
    cv = np.asarray(cost_volume, np.float32).reshape(N_IMG, C_IN, H, W)
    cstb, cstf = _build_consts(
        np.asarray(conv1_w, np.float32), np.asarray(conv1_b, np.float32),
        np.asarray(conv2_w, np.float32), np.asarray(conv2_b, np.float32))

    # halo'd, zero-padded shards: core c = 2*n + h
    sw = np.lib.stride_tricks.sliding_window_view
    in_maps = []
    for n in range(N_IMG):
        cvp = np.zeros((C_IN, H + 2, WP), bf16)
        cvp[:, 1:H + 1, 1:W + 1] = cv[n].astype(bf16)
        dpad = np.zeros((H + 2, WP), np.float32)
        dpad[1:H + 1, 1:W + 1] = depth[n]
        # unfold: du[(r*9 + ky*3 + kx), x] = dpad[r+ky, x+kx]
        win = sw(dpad, (3, W + 2))[:H, 0]                # [H,3,W+2]
        du = np.stack([win[:, :, kx:kx + W] for kx in range(3)], 2)
        du = du.reshape(H * 9, W).astype(bf16)
        for h in range(2):
            r0 = h * HALF
            in_maps.append({
                "xh": np.ascontiguousarray(cvp[:, r0:r0 + HALF + 2, :]),
                "dunf": np.ascontiguousarray(du[9 * r0: 9 * (r0 + HALF)]),
                "cstb": cstb,
                "cstf": cstf,
            })
    return in_maps


def kernel(depth, cost_volume, conv1_w, conv1_b, conv2_w, conv2_b):
    global _NC_CACHE
    from concourse.bass_utils import run_bass_kernel_spmd

    in_maps = prep_inputs(depth, cost_volume, conv1_w, conv1_b,
                          conv2_w, conv2_b)
    if _NC_CACHE is None:
        _NC_CACHE = _build_bass()
    res = run_bass_kernel_spmd(_NC_CACHE, in_maps, core_ids=list(range(8)))
    out = np.empty((N_IMG, 2 * H, 2 * W), np.float32)
    for c, r in enumerate(res.results):
        n, h = c // 2, c % 2
        out[n, 2 * h * HALF: 2 * (h + 1) * HALF, :] = r["out"]
    return out


# revision 8
# speedup vs baseline: 1.0580x; 1.0580x over previous
"""Depth-upsample module kernel for 8 TRN2 NeuronCores.

Pipeline per core (1/8 of batch*height):
  conv1 3x3 8->8 + bias + relu   (PE banded-dy bf16 matmuls, 3 per block)
  conv2 1x1 8->36 (raw)          (PE bf16, 1 matmul per subpixel ab)
  EP = [exp(0.25*conv2+0.25*b2) | E*unfold(depth)]   (ACT exp | DVE mul, bf16)
  [Den|Num] = band @ EP          (PE bf16, 3 chunk matmuls per ab -> psum quad)
  out = Num * recip_approx(Den)  (DVE), b-interleaved in SBUF, contiguous DMA

Layout: row-blocks of R=14 output rows; SBUF partitions pack (row, channel):
  conv input  xb   [(r16,i8)=128, 642]   bf16
  conv1 out   Y    [(r14,o8)=112, 640]   bf16
  EP               [(r14,k9)=126, 1280]  bf16: E cols 0:640, P cols 640:1280
  psumND           [128, 1280] f32: quadrant ab rows 32ab..+14, Den|Num cols
  out interleave   OI[64a+r, 2x+b] f32, rows of 2W -> contiguous DMA
PSUM budget: ps12 pool 2x2 banks (conv1/conv2 rotate) + nd 3 banks = 7 of 8.
"""

import numpy as np
import ml_dtypes

H, W = 512, 640
N_IMG, C_IN = 4, 8
HALF = H // 2           # rows per core (shard = image x half)
RB = 14                 # output rows per block
WP = W + 2              # padded width
CWB = 854               # bf16 const cols: w1(336) w2(504) band(14)
CWF = 5                 # f32 const cols: b1(1) b2(4)


def _build_consts(conv1_w, conv1_b, conv2_w, conv2_b):
    f32 = np.float32
    bf16 = ml_dtypes.bfloat16
    # lhsT1[dx, (r,i), (r',o)] = W1[o,i,r-r',dx] for r-r' in {0,1,2}
    lhsT1 = np.zeros((3, 128, 112), f32)
    for dx in range(3):
        for rp in range(14):
            for dy in range(3):
                r = rp + dy
                lhsT1[dx, r * 8:(r + 1) * 8, rp * 8:(rp + 1) * 8] = \
                    conv1_w[:, :, dy, dx].T  # [i, o]
    # lhsT2[ab, (r,i), (r,k)] = W2[4k+ab, i]
    lhsT2 = np.zeros((4, 112, 126), f32)
    w2 = conv2_w[:, :, 0, 0]  # [36, 8]
    for ab in range(4):
        for r in range(14):
            for k in range(9):
                lhsT2[ab, r * 8:(r + 1) * 8, r * 9 + k] = w2[k * 4 + ab, :]
    # band[(r,k), r'] = 1 iff r == r'
    band = np.zeros((126, 14), f32)
    for r in range(14):
        band[r * 9:(r + 1) * 9, r] = 1
    cstb = np.zeros((128, CWB), bf16)
    for dx in range(3):
        cstb[:, 112 * dx: 112 * (dx + 1)] = lhsT1[dx].astype(bf16)
    for ab in range(4):
        cstb[:112, 336 + 126 * ab: 336 + 126 * (ab + 1)] = lhsT2[ab].astype(bf16)
    cstb[:126, 840:854] = band.astype(bf16)
    cstf = np.zeros((128, CWF), f32)
    cstf[:112, 0] = np.tile(conv1_b.astype(f32), 14)
    for ab in range(4):
        for r in range(14):
            for k in range(9):
                cstf[r * 9 + k, 1 + ab] = 0.25 * float(conv2_b[k * 4 + ab])
    return cstb, cstf


def _build_bass():
    import concourse.bass as bass
    import concourse.bacc as bacc
    import concourse.tile as tile
    from concourse import mybir

    f32 = mybir.dt.float32
    bf16 = mybir.dt.bfloat16
    nc = bacc.Bacc(None, target_bir_lowering=False)

    X = nc.dram_tensor("xh", [C_IN, HALF + 2, WP], bf16, kind="ExternalInput")
    DUNF = nc.dram_tensor("dunf", [HALF * 9, W], bf16, kind="ExternalInput")
    CONSTB = nc.dram_tensor("cstb", [128, CWB], bf16, kind="ExternalInput")
    CONSTF = nc.dram_tensor("cstf", [128, CWF], f32, kind="ExternalInput")
    OUT = nc.dram_tensor("out", [2 * HALF, 2 * W], f32, kind="ExternalOutput")

    nblocks = (HALF + RB - 1) // RB  # 19 (last block R=4)

    with tile.TileContext(nc) as tc:
        with (
            tc.tile_pool(name="consts", bufs=1) as consts,
            tc.tile_pool(name="xp", bufs=3) as xp,
            tc.tile_pool(name="dp", bufs=3) as dp,
            tc.tile_pool(name="yp", bufs=2) as yp,
            tc.tile_pool(name="ep", bufs=10) as ep,
            tc.tile_pool(name="op", bufs=3) as op,
            tc.tile_pool(name="scr", bufs=2) as scr,
            tc.tile_pool(name="ps12", bufs=2, space="PSUM") as ps12,
            tc.tile_pool(name="nd", bufs=1, space="PSUM") as nd,
        ):
            cstb = consts.tile([128, CWB], bf16, tag="cstb")
            nc.sync.dma_start(out=cstb, in_=CONSTB[:])
            cstf = consts.tile([128, CWF], f32, tag="cstf")
            nc.sync.dma_start(out=cstf, in_=CONSTF[:])
            w1t = [cstb[:, 112 * dx: 112 * (dx + 1)] for dx in range(3)]
            w2t = [cstb[:112, 336 + 126 * ab: 336 + 126 * (ab + 1)]
                   for ab in range(4)]
            bandt = cstb[:126, 840:854]
            b1t = cstf[:112, 0:1]
            b2t = [cstf[:126, 1 + ab: 2 + ab] for ab in range(4)]
            # consume the const-DMA ticks (keeps real ops at <=1 wait)
            nc.tensor.ldweights(cstb[:1, :2])
            scrf = scr.tile([1, 1], f32, tag="scrf")
            nc.vector.tensor_copy(scrf, cstf[:1, :1])

            def _rk(b):
                R = min(RB, HALF - RB * b)
                return R, R + 2, RB * b, R * 9, R * 8

            eps = {}  # b -> list of 4 EP tiles (E | P cols)

            def conv_phase(b):
                R, Rin, s, kp, yq = _rk(b)
                # --- load conv input block [(r,i), w] bf16 ---
                xb = xp.tile([128, WP], bf16, tag="xb", name="xb")
                x_in = bass.AP(
                    tensor=X[:].tensor, offset=s * WP,
                    ap=[[WP, Rin], [(HALF + 2) * WP, C_IN], [1, WP]],
                )
                nc.sync.dma_start(out=xb[: Rin * 8], in_=x_in)

                # --- load unfolded depth [(r,k), x] bf16 (host-prepared) ---
                dunf = dp.tile([126, W], bf16, tag="dunf", name="dunf")
                nc.sync.dma_start(out=dunf[:kp], in_=DUNF[9 * s: 9 * s + kp])
                scrap = scr.tile([1, 1], bf16, tag="scrap", name="scrap")
                nc.vector.tensor_copy(scrap, dunf[:1, :1])  # eat DMA tick

                # --- conv1: 3 dx matmuls x 2 col chunks -> psum1 ---
                nc.tensor.ldweights(xb[:1, :2])  # eat DMA tick
                psum1 = ps12.tile([128, W], f32, tag="ps", name="psum1")
                for c0, cn in ((0, 512), (512, 128)):
                    for dx in range(3):
                        nc.tensor.matmul(
                            psum1[:yq, c0:c0 + cn],
                            w1t[dx][: Rin * 8, :yq],
                            xb[: Rin * 8, dx + c0: dx + c0 + cn],
                            start=(dx == 0), stop=(dx == 2),
                        )

                # --- bias+relu -> Y (SBUF bf16) ---
                Y = yp.tile([112, W], bf16, tag="y", name="Y")
                nc.scalar.activation(
                    out=Y[:yq], in_=psum1[:yq],
                    func=mybir.ActivationFunctionType.Relu,
                    bias=b1t[:yq], scale=1.0,
                )
                nc.tensor.ldweights(Y[:1, :2])  # eat ACT tick

                # --- per ab: conv2 -> exp -> x depth -> EP = [E | E*d] ---
                eps[b] = []
                for ab in range(4):
                    psum2 = ps12.tile([128, W], f32, tag="ps", name="psum2")
                    for c0, cn in ((0, 512), (512, 128)):
                        nc.tensor.matmul(
                            psum2[:kp, c0:c0 + cn],
                            w2t[ab][:yq, :kp],
                            Y[:yq, c0:c0 + cn],
                            start=True, stop=True,
                        )
                    EP = ep.tile([126, 2 * W], bf16, tag="ep", name="EP")
                    nc.scalar.activation(
                        out=EP[:kp, 0:W], in_=psum2[:kp],
                        func=mybir.ActivationFunctionType.Exp,
                        bias=b2t[ab][:kp], scale=0.25,
                    )
                    nc.vector.tensor_mul(
                        EP[:kp, W:2 * W], EP[:kp, 0:W], dunf[:kp])
                    eps[b].append(EP)

            def out_phase(b):
                R, Rin, s, kp, yq = _rk(b)
                # --- band-reduce 9 taps: [Den|Num] -> psum quadrant ab ---
                psumND = nd.tile([128, 2 * W], f32, tag="psumnd", name="psumND")
                for ab in range(4):
                    EP = eps[b][ab]
                    for c0, cn in ((0, 512), (512, 512), (1024, 256)):
                        nc.tensor.matmul(
                            psumND[32 * ab: 32 * ab + R, c0:c0 + cn],
                            bandt[:kp, :R], EP[:kp, c0:c0 + cn],
                            start=True, stop=True,
                            tile_position=(0, 32 * ab),
                        )
                del eps[b]

                # --- divide; interleave b into contiguous 2W rows in SBUF ---
                RD = op.tile([128, W], f32, tag="rd", name="RD")
                nc.vector.reciprocal_approx_fast(out=RD, in_=psumND[:, 0:W])
                OI = op.tile([128, 2 * W], f32, tag="oi", name="OI")
                for ab in range(4):
                    a, bb = ab >> 1, ab & 1
                    nc.vector.tensor_mul(
                        OI[64 * a: 64 * a + R, bb::2],
                        psumND[32 * ab: 32 * ab + R, W:2 * W],
                        RD[32 * ab: 32 * ab + R])

                # --- store: out[2(s+r)+a, :] = OI[64a+r, :]  (contiguous rows)
                for a in range(2):
                    o_out = bass.AP(
                        tensor=OUT[:].tensor,
                        offset=(2 * s + a) * (2 * W),
                        ap=[[4 * W, R], [1, 2 * W]],
                    )
                    nc.sync.dma_start(out=o_out, in_=OI[64 * a: 64 * a + R])

            # software pipeline: band/output of block b-1 is emitted after
            # the conv phase of block b, giving PE a full phase of distance
            # between EP production and consumption.
            conv_phase(0)
            for b in range(1, nblocks):
                conv_phase(b)
                out_phase(b - 1)
            out_phase(nblocks - 1)

    nc.compile()
    return nc


_NC_CACHE = None


def prep_inputs(depth, cost_volume, conv1_w, conv1_b, conv2_w, conv2_b):
    bf16 = ml_dtypes.bfloat16
    depth = np.asarray(depth, np.float32)
    cv = np.asarray(cost_volume, np.float32).reshape(N_IMG, C_IN, H, W)
    cstb, cstf = _build_consts(
        np.asarray(conv1_w, np.float32), np.asarray(conv1_b, np.float32),
        np.asarray(conv2_w, np.float32), np.asarray(conv2_b, np.float32))

    # halo'd, zero-padded shards: core c = 2*n + h
    sw = np.lib.stride_tricks.sliding_window_view
    in_maps = []
    for n in range(N_IMG):
        cvp = np.zeros((C_IN, H + 2, WP), bf16)
        cvp[:, 1:H + 1, 1:W + 1] = cv[n].astype(bf16)
        dpad = np.zeros((H + 2, WP), np.float32)
        dpad[1:H + 1, 1:W + 1] = depth[n]
        # unfold: du[(r*9 + ky*3 + kx), x] = dpad[r+ky, x+kx]
        win = sw(dpad, (3, W + 2))[:H, 0]                # [H,3,W+2]
        du = np.stack([win[:, :, kx:kx + W] for kx in range(3)], 2)
        du = du.reshape(H * 9, W).astype(bf16)
        for h in range(2):
            r0 = h * HALF
            in_maps.append({
                "xh": np.ascontiguousarray(cvp[:, r0:r0 + HALF + 2, :]),
                "dunf": np.ascontiguousarray(du[9 * r0: 9 * (r0 + HALF)]),
                "cstb": cstb,
                "cstf": cstf,
            })
    return in_maps


def kernel(depth, cost_volume, conv1_w, conv1_b, conv2_w, conv2_b):
    global _NC_CACHE
    from concourse.bass_utils import run_bass_kernel_spmd

    in_maps = prep_inputs(depth, cost_volume, conv1_w, conv1_b,
                          conv2_w, conv2_b)
    if _NC_CACHE is None:
        _NC_CACHE = _build_bass()
    res = run_bass_kernel_spmd(_NC_CACHE, in_maps, core_ids=list(range(8)))
    out = np.empty((N_IMG, 2 * H, 2 * W), np.float32)
    for c, r in enumerate(res.results):
        n, h = c // 2, c % 2
        out[n, 2 * h * HALF: 2 * (h + 1) * HALF, :] = r["out"]
    return out


# revision 11
# speedup vs baseline: 1.4873x; 1.4057x over previous
"""Depth-upsample module kernel for 8 TRN2 NeuronCores.

Pipeline per core (1/8 of batch*height):
  conv1 3x3 8->8 + bias + relu   (PE banded-dy bf16 matmuls, 3 per block)
  conv2 1x1 8->36 (raw)          (PE bf16, 1 matmul per subpixel ab)
  EP = [exp(0.25*conv2+0.25*b2) | E*unfold(depth)]   (ACT exp | DVE mul, bf16)
  [Den|Num] = band @ EP          (PE bf16, 3 chunk matmuls per ab -> psum quad)
  out = Num * recip_approx(Den)  (DVE), b-interleaved in SBUF, contiguous DMA

Layout: row-blocks of R=14 output rows; SBUF partitions pack (row, channel):
  conv input  xb   [(r16,i8)=128, 642]   bf16
  conv1 out   Y    [(r14,o8)=112, 640]   bf16
  EP               [(r14,k9)=126, 1280]  bf16: E cols 0:640, P cols 640:1280
  psumND           [128, 1280] f32: quadrant ab rows 32ab..+14, Den|Num cols
  out interleave   OI[64a+r, 2x+b] f32, rows of 2W -> contiguous DMA
PSUM budget: ps12 pool 2x2 banks (conv1/conv2 rotate) + nd 3 banks = 7 of 8.
"""

import numpy as np
import ml_dtypes

H, W = 512, 640
N_IMG, C_IN = 4, 8
HALF = H // 2           # rows per core (shard = image x half)
RB = 14                 # output rows per block
WP = W + 2              # padded width
CWB = 854               # bf16 const cols: w1(336) w2(504) band(14)
CWF = 5                 # f32 const cols: b1(1) b2(4)


def _build_consts(conv1_w, conv1_b, conv2_w, conv2_b):
    f32 = np.float32
    bf16 = ml_dtypes.bfloat16
    # lhsT1[dx, (r,i), (r',o)] = W1[o,i,r-r',dx] for r-r' in {0,1,2}
    lhsT1 = np.zeros((3, 128, 112), f32)
    for dx in range(3):
        for rp in range(14):
            for dy in range(3):
                r = rp + dy
                lhsT1[dx, r * 8:(r + 1) * 8, rp * 8:(rp + 1) * 8] = \
                    conv1_w[:, :, dy, dx].T  # [i, o]
    # lhsT2[ab, (r,i), (r,k)] = W2[4k+ab, i]
    lhsT2 = np.zeros((4, 112, 126), f32)
    w2 = conv2_w[:, :, 0, 0]  # [36, 8]
    for ab in range(4):
        for r in range(14):
            for k in range(9):
                lhsT2[ab, r * 8:(r + 1) * 8, r * 9 + k] = w2[k * 4 + ab, :]
    # band[(r,k), r'] = 1 iff r == r'
    band = np.zeros((126, 14), f32)
    for r in range(14):
        band[r * 9:(r + 1) * 9, r] = 1
    cstb = np.zeros((128, CWB), bf16)
    for dx in range(3):
        cstb[:, 112 * dx: 112 * (dx + 1)] = lhsT1[dx].astype(bf16)
    for ab in range(4):
        cstb[:112, 336 + 126 * ab: 336 + 126 * (ab + 1)] = lhsT2[ab].astype(bf16)
    cstb[:126, 840:854] = band.astype(bf16)
    cstf = np.zeros((128, CWF), f32)
    cstf[:112, 0] = np.tile(conv1_b.astype(f32), 14)
    for ab in range(4):
        for r in range(14):
            for k in range(9):
                cstf[r * 9 + k, 1 + ab] = 0.25 * float(conv2_b[k * 4 + ab])
    return cstb, cstf


def _build_bass():
    import concourse.bass as bass
    import concourse.bacc as bacc
    import concourse.tile as tile
    from concourse import mybir

    f32 = mybir.dt.float32
    bf16 = mybir.dt.bfloat16
    nc = bacc.Bacc(None, target_bir_lowering=False)

    X = nc.dram_tensor("xh", [C_IN, HALF + 2, WP], bf16, kind="ExternalInput")
    DUNF = nc.dram_tensor("dunf", [HALF * 9, W], bf16, kind="ExternalInput")
    CONSTB = nc.dram_tensor("cstb", [128, CWB], bf16, kind="ExternalInput")
    CONSTF = nc.dram_tensor("cstf", [128, CWF], f32, kind="ExternalInput")
    # raw quadrant planes [ab, r, x]; host interleaves to [2r+a, 2x+b]
    OUT = nc.dram_tensor("out", [4, HALF, W], f32, kind="ExternalOutput")

    nblocks = (HALF + RB - 1) // RB  # 19 (last block R=4)

    with tile.TileContext(nc) as tc:
        with (
            tc.tile_pool(name="consts", bufs=1) as consts,
            tc.tile_pool(name="xp", bufs=3) as xp,
            tc.tile_pool(name="dp", bufs=3) as dp,
            tc.tile_pool(name="yp", bufs=2) as yp,
            tc.tile_pool(name="ep", bufs=10) as ep,
            tc.tile_pool(name="op", bufs=3) as op,
            tc.tile_pool(name="scr", bufs=2) as scr,
            tc.tile_pool(name="ps12", bufs=2, space="PSUM") as ps12,
            tc.tile_pool(name="nd", bufs=1, space="PSUM") as nd,
        ):
            cstb = consts.tile([128, CWB], bf16, tag="cstb")
            nc.sync.dma_start(out=cstb, in_=CONSTB[:])
            cstf = consts.tile([128, CWF], f32, tag="cstf")
            nc.sync.dma_start(out=cstf, in_=CONSTF[:])
            w1t = [cstb[:, 112 * dx: 112 * (dx + 1)] for dx in range(3)]
            w2t = [cstb[:112, 336 + 126 * ab: 336 + 126 * (ab + 1)]
                   for ab in range(4)]
            bandt = cstb[:126, 840:854]
            b1t = cstf[:112, 0:1]
            b2t = [cstf[:126, 1 + ab: 2 + ab] for ab in range(4)]
            # consume the const-DMA ticks (keeps real ops at <=1 wait)
            nc.tensor.ldweights(cstb[:1, :2])
            scrf = scr.tile([1, 1], f32, tag="scrf")
            nc.vector.tensor_copy(scrf, cstf[:1, :1])

            def _rk(b):
                R = min(RB, HALF - RB * b)
                return R, R + 2, RB * b, R * 9, R * 8

            eps = {}  # b -> list of 4 EP tiles (E | P cols)

            def conv_phase(b):
                R, Rin, s, kp, yq = _rk(b)
                # --- load conv input block [(r,i), w] bf16 ---
                xb = xp.tile([128, WP], bf16, tag="xb", name="xb")
                x_in = bass.AP(
                    tensor=X[:].tensor, offset=s * WP,
                    ap=[[WP, Rin], [(HALF + 2) * WP, C_IN], [1, WP]],
                )
                nc.sync.dma_start(out=xb[: Rin * 8], in_=x_in)

                # --- load unfolded depth [(r,k), x] bf16 (host-prepared) ---
                dunf = dp.tile([126, W], bf16, tag="dunf", name="dunf")
                nc.sync.dma_start(out=dunf[:kp], in_=DUNF[9 * s: 9 * s + kp])
                scrap = scr.tile([1, 1], bf16, tag="scrap", name="scrap")
                nc.vector.tensor_copy(scrap, dunf[:1, :1])  # eat DMA tick

                # --- conv1: 3 dx matmuls x 2 col chunks -> psum1 ---
                nc.tensor.ldweights(xb[:1, :2])  # eat DMA tick
                psum1 = ps12.tile([128, W], f32, tag="ps", name="psum1")
                for c0, cn in ((0, 512), (512, 128)):
                    for dx in range(3):
                        nc.tensor.matmul(
                            psum1[:yq, c0:c0 + cn],
                            w1t[dx][: Rin * 8, :yq],
                            xb[: Rin * 8, dx + c0: dx + c0 + cn],
                            start=(dx == 0), stop=(dx == 2),
                        )

                # --- bias+relu -> Y (SBUF bf16) ---
                Y = yp.tile([112, W], bf16, tag="y", name="Y")
                nc.scalar.activation(
                    out=Y[:yq], in_=psum1[:yq],
                    func=mybir.ActivationFunctionType.Relu,
                    bias=b1t[:yq], scale=1.0,
                )
                nc.tensor.ldweights(Y[:1, :2])  # eat ACT tick

                # --- per ab: conv2 -> exp -> x depth -> EP = [E | E*d] ---
                eps[b] = []
                for ab in range(4):
                    psum2 = ps12.tile([128, W], f32, tag="ps", name="psum2")
                    for c0, cn in ((0, 512), (512, 128)):
                        nc.tensor.matmul(
                            psum2[:kp, c0:c0 + cn],
                            w2t[ab][:yq, :kp],
                            Y[:yq, c0:c0 + cn],
                            start=True, stop=True,
                        )
                    EP = ep.tile([126, 2 * W], bf16, tag="ep", name="EP")
                    nc.scalar.activation(
                        out=EP[:kp, 0:W], in_=psum2[:kp],
                        func=mybir.ActivationFunctionType.Exp,
                        bias=b2t[ab][:kp], scale=0.25,
                    )
                    nc.vector.tensor_mul(
                        EP[:kp, W:2 * W], EP[:kp, 0:W], dunf[:kp])
                    eps[b].append(EP)

            def out_phase(b):
                R, Rin, s, kp, yq = _rk(b)
                # --- band-reduce 9 taps: [Den|Num] -> psum quadrant ab ---
                psumND = nd.tile([128, 2 * W], f32, tag="psumnd", name="psumND")
                for ab in range(4):
                    EP = eps[b][ab]
                    for c0, cn in ((0, 512), (512, 512), (1024, 256)):
                        nc.tensor.matmul(
                            psumND[32 * ab: 32 * ab + R, c0:c0 + cn],
                            bandt[:kp, :R], EP[:kp, c0:c0 + cn],
                            start=True, stop=True,
                            tile_position=(0, 32 * ab),
                        )
                del eps[b]

                # --- divide: one op over all 4 quadrants (partition-parallel)
                RD = op.tile([128, W], f32, tag="rd", name="RD")
                nc.vector.reciprocal_approx_fast(out=RD, in_=psumND[:, 0:W])
                O = op.tile([128, W], f32, tag="o", name="O")
                nc.vector.tensor_mul(O, psumND[:, W:2 * W], RD)

                # --- store quadrant planes: out[ab, s+r, :] = O[32ab+r, :]
                for ab in range(4):
                    o_out = bass.AP(
                        tensor=OUT[:].tensor,
                        offset=(ab * HALF + s) * W,
                        ap=[[W, R], [1, W]],
                    )
                    nc.sync.dma_start(out=o_out, in_=O[32 * ab: 32 * ab + R])

            # software pipeline: band/output of block b-1 is emitted after
            # the conv phase of block b, giving PE a full phase of distance
            # between EP production and consumption.
            conv_phase(0)
            for b in range(1, nblocks):
                conv_phase(b)
                out_phase(b - 1)
            out_phase(nblocks - 1)

    nc.compile()
    return nc


_NC_CACHE = None


def prep_inputs(depth, cost_volume, conv1_w, conv1_b, conv2_w, conv2_b):
    bf16 = ml_dtypes.bfloat16
    depth = np.asarray(depth, np.float32)
    cv = np.asarray(cost_volume, np.float32).reshape(N_IMG, C_IN, H, W)
    cstb, cstf = _build_consts(
        np.asarray(conv1_w, np.float32), np.asarray(conv1_b, np.float32),
        np.asarray(conv2_w, np.float32), np.asarray(conv2_b, np.float32))

    # halo'd, zero-padded shards: core c = 2*n + h
    sw = np.lib.stride_tricks.sliding_window_view
    in_maps = []
    for n in range(N_IMG):
        cvp = np.zeros((C_IN, H + 2, WP), bf16)
        cvp[:, 1:H + 1, 1:W + 1] = cv[n].astype(bf16)
        dpad = np.zeros((H + 2, WP), np.float32)
        dpad[1:H + 1, 1:W + 1] = depth[n]
        # unfold: du[(r*9 + ky*3 + kx), x] = dpad[r+ky, x+kx]
        win = sw(dpad, (3, W + 2))[:H, 0]                # [H,3,W+2]
        du = np.stack([win[:, :, kx:kx + W] for kx in range(3)], 2)
        du = du.reshape(H * 9, W).astype(bf16)
        for h in range(2):
            r0 = h * HALF
            in_maps.append({
                "xh": np.ascontiguousarray(cvp[:, r0:r0 + HALF + 2, :]),
                "dunf": np.ascontiguousarray(du[9 * r0: 9 * (r0 + HALF)]),
                "cstb": cstb,
                "cstf": cstf,
            })
    return in_maps


def kernel(depth, cost_volume, conv1_w, conv1_b, conv2_w, conv2_b):
    global _NC_CACHE
    from concourse.bass_utils import run_bass_kernel_spmd

    in_maps = prep_inputs(depth, cost_volume, conv1_w, conv1_b,
                          conv2_w, conv2_b)
    if _NC_CACHE is None:
        _NC_CACHE = _build_bass()
    res = run_bass_kernel_spmd(_NC_CACHE, in_maps, core_ids=list(range(8)))
    out = np.empty((N_IMG, 2 * H, 2 * W), np.float32)
    for c, r in enumerate(res.results):
        n, h = c // 2, c % 2
        # raw[2a+b, r, x] -> out[2r+a, 2x+b]
        raw = r["out"].reshape(2, 2, HALF, W)
        out[n, 2 * h * HALF: 2 * (h + 1) * HALF, :] = \
            raw.transpose(2, 0, 3, 1).reshape(2 * HALF, 2 * W)
    return out
